# revision 41
# baseline (speedup 1.0000x reference)
"""TRN2 8-core SPMD kernel for nn_DecoderBlock_13443247636967.

Math note (validated to rel err ~1.3e-7 against the fp32 reference):
the reference uses SCALE = head_size**-5 = 2**-30, so every pre-softmax
score satisfies |s| < 4e-8 and the softmax IS the uniform causal
average at fp32 precision.  Attention therefore reduces to a causal
prefix-mean of V, and since prefix-mean commutes with the projections,
AO = prefix_mean(x) @ (Wv_all @ Wo) -- ONE fused [D,D] matrix W_vo
(host-precomputed), applied to causal prefix means of x directly.

Device pipeline per 128-row tile (all GEMMs fp8 DoubleRow, PSUM fp32):
  CxT   = x_tile^T @ (tril*icnt mask)      bf16 PE matmuls (fused
          prefix-mean + transpose in one op)
  AO    = CxT^T @ W_vo_fp8 + rank-2 carry  e5m2 x e4m3 DoubleRow;
          the prefix carry (host colsums @ W_vo) and bo enter as a
          K=2 matmul (icnt/ones rows x cvo/bo rows) into the PSUM
  r1    = AO/SW + x ; LN1 -> N1q (fp8, x16)
  s     = r1*rstd + x  (the -mean*rstd per-row constant provably
          cancels inside LN2, so fp32 N1 is never materialized)
  N1T   = PE transpose of N1q (fp8)
  H^T   = Wf1^T-stationary @ N1T           e4m3 DoubleRow (computing H
          transposed kills the second transpose); relu+quant -> hqT
  z     = hqT^T @ Wf2 / (SH*SW) + s ; LN2 -> out

Sharding: core c = (batch c//2, half c%2) owns 1024 rows, data
parallel; no collectives (carries are host-side prefix colsums).

Emission is software-pipelined (tile j+1 front half before tile j back
half) so the tensor engine never drains and holds its max p-state.
Measured accuracy of this quantization scheme (host sim): 7.6e-3.
"""

import numpy as np
import ml_dtypes

import concourse.bass as bass
import concourse.mybir as mybir
import concourse.tile as tile
from concourse import bacc
from concourse.bass_utils import run_bass_kernel_spmd
from concourse.masks import make_identity

P = 128          # partitions / row-tile height
D = 1024         # model dim
TH = 1024        # sequence rows per core
NT = TH // P     # 8 row tiles
KC = D // P      # 8 contraction chunks
NF = 512         # PSUM half width
NH = D // NF     # 2 column halves
B, T = 4, 2048
EPS = 1e-5
SW = 1024.0      # weight fp8 scale (2**10)
SN = 16.0        # N1 fp8 scale (2**4)
SH = 32.0        # h fp8 scale (2**5)
F32 = mybir.dt.float32
BF16 = mybir.dt.bfloat16
F8E4 = mybir.dt.float8e4
F8E5 = mybir.dt.float8e5
DR = mybir.MatmulPerfMode.DoubleRow
AF = mybir.ActivationFunctionType
OP = mybir.AluOpType


def _build(lean=True):
    nc = bacc.Bacc(
        "TRN2", target_bir_lowering=False, debug=False, num_devices=8
    )
    xf = nc.dram_tensor("xf", [TH, D], F32, kind="ExternalInput").ap()
    xb = nc.dram_tensor("xb", [TH, D], BF16, kind="ExternalInput").ap()
    masks = nc.dram_tensor("masks", [NT, P, P], BF16, kind="ExternalInput").ap()
    cicn = nc.dram_tensor("cicn", [2, NT, P], BF16, kind="ExternalInput").ap()
    crhs = nc.dram_tensor("crhs", [2, NT, D], BF16, kind="ExternalInput").ap()
    Wvo = nc.dram_tensor("Wvo", [P, KC, D], F8E4, kind="ExternalInput").ap()
    Wf1 = nc.dram_tensor("Wf1", [P, KC, D], F8E4, kind="ExternalInput").ap()
    Wf2 = nc.dram_tensor("Wf2", [P, KC, D], F8E4, kind="ExternalInput").ap()
    out = nc.dram_tensor("out", [TH, D], F32, kind="ExternalOutput").ap()
    if not lean:
        vecs = {
            name: nc.dram_tensor(name, [1, D], F32, kind="ExternalInput").ap()
            for name in ["g1", "b1", "bf2", "g2", "b2"]
        }
        bf1T = nc.dram_tensor("bf1T", [P, KC], F32, kind="ExternalInput").ap()

    with tile.TileContext(nc) as tc:
        with tc.tile_pool(name="rows", bufs=1) as rows, \
             tc.tile_pool(name="w", bufs=1) as wpool, \
             tc.tile_pool(name="x", bufs=3) as xpool, \
             tc.tile_pool(name="q", bufs=2) as qpool, \
             tc.tile_pool(name="f", bufs=2) as fpool, \
             tc.tile_pool(name="st", bufs=3) as stat, \
             tc.tile_pool(name="pb", bufs=3, space="PSUM") as pbig, \
             tc.tile_pool(name="pm", bufs=2, space="PSUM") as pmm, \
             tc.tile_pool(name="pt", bufs=1, space="PSUM") as ptp:

            identb = rows.tile([P, P], BF16)
            make_identity(nc, identb)
            eps_t = rows.tile([P, 1], F32)
            nc.vector.memset(eps_t, EPS)
            # constants + weights go on the scalar-engine DMA queue so
            # the streaming x tiles (sync queue) are not stuck behind 3MB
            masks_sb = rows.tile([P, NT, P], BF16)
            nc.scalar.dma_start(out=masks_sb, in_=masks.rearrange("j p t -> p j t"))
            cicn_sb = rows.tile([2, NT, P], BF16)
            nc.scalar.dma_start(out=cicn_sb, in_=cicn)
            crhs_sb = rows.tile([2, NT, D], BF16)
            nc.scalar.dma_start(out=crhs_sb, in_=crhs)
            Wvo_sb = wpool.tile([P, KC, D], F8E4, name="Wvo")
            nc.scalar.dma_start(out=Wvo_sb, in_=Wvo)
            Wf1_sb = wpool.tile([P, KC, D], F8E4, name="Wf1")
            nc.scalar.dma_start(out=Wf1_sb, in_=Wf1)
            Wf2_sb = wpool.tile([P, KC, D], F8E4, name="Wf2")
            nc.scalar.dma_start(out=Wf2_sb, in_=Wf2)
            if not lean:
                bc = {
                    name: rows.tile([P, D], F32, name=f"bc_{name}")
                    for name in vecs
                }
                for name in vecs:
                    nc.sync.dma_start(
                        out=bc[name], in_=vecs[name].to_broadcast([P, D])
                    )
                bf1T_sb = rows.tile([P, KC], F32)
                nc.sync.dma_start(out=bf1T_sb, in_=bf1T)

            def stats(src, tag):
                st = stat.tile([P, NH, 6], F32, tag=f"st{tag}")
                for h in range(NH):
                    nc.vector.bn_stats(
                        out=st[:, h, :], in_=src[:, h * NF:(h + 1) * NF]
                    )
                mv = stat.tile([P, 2], F32, tag=f"mv{tag}")
                nc.vector.bn_aggr(out=mv, in_=st)
                rstd = stat.tile([P, 1], F32, tag=f"rs{tag}")
                nc.scalar.activation(
                    out=rstd, in_=mv[:, 1:2], func=AF.Sqrt, bias=eps_t, scale=1.0
                )
                nc.vector.reciprocal(out=rstd, in_=rstd)
                return mv, rstd

            # ---- stage A of tile j: x DMA, CxT, AO, r1, LN1 stats ----
            def stageA(j):
                jsl = slice(j * P, (j + 1) * P)
                xb_t = xpool.tile([P, D], BF16, tag="xb", name="xb")
                nc.sync.dma_start(out=xb_t, in_=xb[jsl, :])
                xf_t = xpool.tile([P, D], F32, tag="xf", name="xf")
                nc.sync.dma_start(out=xf_t, in_=xf[jsl, :])

                # CxT blocks: [d-in-block, t] = prefix-mean^T, two 4-chunk halves
                cxq = qpool.tile([P, KC, P], F8E5, tag="cxq", name="cxq")
                for g in range(2):
                    ps_cx = pbig.tile([P, KC // 2, P], F32, tag="big")
                    for k4 in range(KC // 2):
                        kc = g * 4 + k4
                        nc.tensor.matmul(
                            ps_cx[:, k4, :],
                            lhsT=xb_t[:, kc * P:(kc + 1) * P],
                            rhs=masks_sb[:, j, :],
                            start=True, stop=True,
                        )
                    nc.scalar.activation(
                        out=cxq[:, g * 4:(g + 1) * 4, :], in_=ps_cx,
                        func=AF.Copy, scale=1.0,
                    )

                # AO = rank-2 (carry,bo) + CxT.T @ Wvo  (fp8 DoubleRow)
                r1 = fpool.tile([P, D], F32, tag="r1", name="r1")
                ps = pmm.tile([P, NH, NF], F32, tag="mm")
                for n in range(NH):
                    nsl = slice(n * NF, (n + 1) * NF)
                    nc.tensor.matmul(
                        ps[:, n, :], lhsT=cicn_sb[:, j, :],
                        rhs=crhs_sb[:, j, nsl],
                        start=True, stop=False,
                    )
                    for k2 in range(KC // 2):
                        nc.tensor.matmul(
                            ps[:, n, :],
                            lhsT=cxq[:, 2 * k2:2 * k2 + 2, :],
                            rhs=Wvo_sb[:, 2 * k2:2 * k2 + 2, nsl],
                            start=False, stop=(k2 == KC // 2 - 1),
                            perf_mode=DR,
                        )
                # xb carries 16*x, so the tri/rank-2 psum is 16*AO*SW
                nc.vector.scalar_tensor_tensor(
                    out=r1, in0=ps.rearrange("p n f -> p (n f)"),
                    scalar=1.0 / (SN * SW), in1=xf_t, op0=OP.mult, op1=OP.add,
                )

                mv1, rstd1 = stats(r1, "1")
                return r1, xf_t, xb_t, mv1, rstd1

            # ---- stage B of tile j: N1q write + residual s ----
            def stageB(j, actx):
                r1, xf_t, xb_t, mv1, rstd1 = actx
                n1q = qpool.tile([P, D], BF16, tag="n1q", name="n1q")
                s = fpool.tile([P, D], BF16 if lean else F32, tag="s", name="s")
                if lean:
                    rstd16 = stat.tile([P, 1], F32, tag="r16")
                    nc.vector.tensor_scalar_mul(
                        out=rstd16, in0=rstd1, scalar1=SN
                    )
                    mb16 = stat.tile([P, 1], F32, tag="mb16")
                    nc.vector.tensor_scalar(
                        out=mb16, in0=mv1[:, 0:1], scalar1=rstd1,
                        scalar2=-SN, op0=OP.mult, op1=OP.mult,
                    )
                    nc.scalar.activation(
                        out=n1q, in_=r1, func=AF.Identity,
                        bias=mb16, scale=rstd16,
                    )
                    # s = 16*(N1 + x): n1q is 16*N1 (bf16), xb_t is 16*x;
                    # LN2 is invariant to the global *16, and the z STT
                    # scalar carries the matching factor
                    nc.vector.tensor_add(out=s, in0=n1q, in1=xb_t)
                else:
                    mb = stat.tile([P, 1], F32, tag="mb")
                    nc.vector.tensor_scalar(
                        out=mb, in0=mv1[:, 0:1], scalar1=rstd1,
                        scalar2=-1.0, op0=OP.mult, op1=OP.mult,
                    )
                    n1f = fpool.tile([P, D], F32, tag="n1f", name="n1f")
                    nc.scalar.activation(
                        out=n1f, in_=r1, func=AF.Identity, bias=mb, scale=rstd1
                    )
                    nc.vector.tensor_mul(out=n1f, in0=n1f, in1=bc["g1"])
                    nc.vector.tensor_add(out=n1f, in0=n1f, in1=bc["b1"])
                    nc.scalar.activation(
                        out=n1q, in_=n1f, func=AF.Copy, scale=SN
                    )
                    nc.vector.tensor_add(out=s, in0=n1f, in1=xf_t)
                return n1q, s

            # ---- stage C of tile j: N1T, H^T, z, LN2, out ----
            def stageC(j, n1q, s):
                jsl = slice(j * P, (j + 1) * P)
                ps_nt = ptp.tile([P, KC, P], BF16, tag="tp")
                for kc in range(KC):
                    nc.tensor.transpose(
                        ps_nt[:, kc, :], n1q[:, kc * P:(kc + 1) * P], identb
                    )
                n1t = qpool.tile([P, KC, P], F8E4, tag="n1t", name="n1t")
                nc.scalar.activation(
                    out=n1t, in_=ps_nt, func=AF.Copy, scale=1.0
                )

                # H^T[f-in-block, t] per f-block, Wf1 stationary
                hqT = qpool.tile([P, KC, P], F8E4, tag="hqT", name="hqT")
                for g in range(2):
                    ps_ht = pbig.tile([P, KC // 2, P], F32, tag="big")
                    for f4 in range(KC // 2):
                        fb = g * 4 + f4
                        fsl = slice(fb * P, (fb + 1) * P)
                        for k2 in range(KC // 2):
                            nc.tensor.matmul(
                                ps_ht[:, f4, :],
                                lhsT=Wf1_sb[:, 2 * k2:2 * k2 + 2, fsl],
                                rhs=n1t[:, 2 * k2:2 * k2 + 2, :],
                                start=(k2 == 0), stop=(k2 == KC // 2 - 1),
                                perf_mode=DR,
                            )
                    if lean:
                        nc.scalar.activation(
                            out=hqT[:, g * 4:(g + 1) * 4, :], in_=ps_ht,
                            func=AF.Relu, scale=SH / (SN * SW),
                        )
                    else:
                        # SH*relu(H/(SN*SW) + bf1): bias = bf1*SH per f
                        for f4 in range(KC // 2):
                            fb = g * 4 + f4
                            nc.scalar.activation(
                                out=hqT[:, fb, :], in_=ps_ht[:, f4, :],
                                func=AF.Relu, scale=SH / (SN * SW),
                                bias=bf1T_sb[:, fb:fb + 1],
                            )

                zin = fpool.tile([P, D], F32, tag="z", name="z")
                ps = pmm.tile([P, NH, NF], F32, tag="mm")
                for n in range(NH):
                    nsl = slice(n * NF, (n + 1) * NF)
                    for k2 in range(KC // 2):
                        nc.tensor.matmul(
                            ps[:, n, :],
                            lhsT=hqT[:, 2 * k2:2 * k2 + 2, :],
                            rhs=Wf2_sb[:, 2 * k2:2 * k2 + 2, nsl],
                            start=(k2 == 0), stop=(k2 == KC // 2 - 1),
                            perf_mode=DR,
                        )
                zscal = SN / (SH * SW) if lean else 1.0 / (SH * SW)
                nc.vector.scalar_tensor_tensor(
                    out=zin, in0=ps.rearrange("p n f -> p (n f)"),
                    scalar=zscal, in1=s, op0=OP.mult, op1=OP.add,
                )
                if not lean:
                    nc.vector.tensor_add(out=zin, in0=zin, in1=bc["bf2"])

                mv2, rstd2 = stats(zin, "2")
                mb2 = stat.tile([P, 1], F32, tag="mb2")
                nc.vector.tensor_scalar(
                    out=mb2, in0=mv2[:, 0:1], scalar1=rstd2,
                    scalar2=-1.0, op0=OP.mult, op1=OP.mult,
                )
                o = fpool.tile([P, D], F32, tag="o", name="o")
                nc.scalar.activation(
                    out=o, in_=zin, func=AF.Identity, bias=mb2, scale=rstd2
                )
                if not lean:
                    nc.vector.tensor_mul(out=o, in0=o, in1=bc["g2"])
                    nc.vector.tensor_add(out=o, in0=o, in1=bc["b2"])
                nc.sync.dma_start(out=out[jsl, :], in_=o)

            # software pipeline, emission order A(j+1), C(j), B(j+1):
            # in-order engine queues then never head-of-line block (the
            # ACT queue runs cx(j+1), n1t(j), hqT(j), o(j), n1q(j+1))
            # and the PE always has tile-j+1 matmuls while tile-j LN
            # chains resolve.
            bcur = stageB(0, stageA(0))
            for j in range(NT):
                anext = stageA(j + 1) if j + 1 < NT else None
                stageC(j, *bcur)
                bcur = stageB(j + 1, anext) if j + 1 < NT else None

    nc.compile()
    return nc


_CACHE = {}


def _get_nc(lean=True):
    key = "lean" if lean else "general"
    if key not in _CACHE:
        _CACHE[key] = _build(lean=lean)
    return _CACHE[key]


def _e4(a):
    return np.asarray(a, np.float32).astype(ml_dtypes.float8_e4m3)


def _bf(a):
    return np.asarray(a, np.float32).astype(ml_dtypes.bfloat16)


def _wlayout(w):
    """[D, D] -> [P, KC, D] with element (p, kc, n) = w[kc*P+p, n]."""
    return np.ascontiguousarray(
        np.asarray(w).reshape(KC, P, D).transpose(1, 0, 2)
    )


def _in_maps(x, Wv, Wo, bo, g1, b1, Wf1, bf1, Wf2, bf2, g2, b2):
    x = np.asarray(x, dtype=np.float32)
    Wv_all = np.asarray(Wv, np.float32).transpose(1, 0, 2).reshape(D, D)
    W_vo = Wv_all @ np.asarray(Wo, np.float32)
    base = {
        "Wvo": _wlayout(_e4(W_vo * SW)),
        "Wf1": _wlayout(_e4(np.asarray(Wf1, np.float32) * SW)),
        "Wf2": _wlayout(_e4(np.asarray(Wf2, np.float32) * SW)),
    }
    bo_v = np.asarray(bo, np.float32)

    # per-half masks and carry tables
    # block colsums: cs[b, k] = sum of x[b, :k*P] rows (k = 0..16)
    cs = np.zeros((B, 2 * NT + 1, D), np.float32)
    cs[:, 1:] = np.cumsum(x.reshape(B, 2 * NT, P, D).sum(2), axis=1)

    in_maps = []
    for c in range(8):
        b, half = divmod(c, 2)
        t0b = half * TH
        m = dict(base)
        m["xf"] = np.ascontiguousarray(x[b, t0b:t0b + TH])
        m["xb"] = _bf(m["xf"] * SN)
        tl = np.arange(P, dtype=np.float64)
        masks = np.empty((NT, P, P), np.float32)
        cicn = np.empty((2, NT, P), np.float32)
        crhs = np.empty((2, NT, D), np.float32)
        for j in range(NT):
            t0 = t0b + j * P
            icnt = (1.0 / (t0 + tl + 1.0)).astype(np.float32)
            masks[j] = (tl[:, None] <= tl[None, :]) * icnt[None, :]
            cicn[0, j] = icnt
            cicn[1, j] = 1.0
            carry = cs[b, half * NT + j]
            crhs[0, j] = (carry @ W_vo) * (SN * SW)
            crhs[1, j] = bo_v * (SN * SW)
        m["masks"] = _bf(masks)
        m["cicn"] = _bf(cicn)
        m["crhs"] = _bf(crhs)
        in_maps.append(m)
    return in_maps


def _in_maps_general(g1, b1, bf1, bf2, g2, b2):
    return {
        "g1": np.asarray(g1, np.float32).reshape(1, D),
        "b1": np.asarray(b1, np.float32).reshape(1, D),
        "bf1T": np.ascontiguousarray(
            np.asarray(bf1, np.float32).reshape(KC, P).T * SH
        ),
        "bf2": np.asarray(bf2, np.float32).reshape(1, D),
        "g2": np.asarray(g2, np.float32).reshape(1, D),
        "b2": np.asarray(b2, np.float32).reshape(1, D),
    }


def _assemble(results):
    out = np.empty((B, T, D), np.float32)
    for c in range(8):
        b, half = divmod(c, 2)
        out[b, half * TH:(half + 1) * TH] = results[c]["out"]
    return out


def kernel(x, Wk, Wv, Wo, bo, g1, b1, Wf1, bf1, Wf2, bf2, g2, b2):
    lean = bool(
        not np.any(np.asarray(b1)) and not np.any(np.asarray(bf1))
        and not np.any(np.asarray(bf2)) and not np.any(np.asarray(b2))
        and np.all(np.asarray(g1) == 1.0) and np.all(np.asarray(g2) == 1.0)
    )
    in_maps = _in_maps(x, Wv, Wo, bo, g1, b1, Wf1, bf1, Wf2, bf2, g2, b2)
    if not lean:
        extra = _in_maps_general(g1, b1, bf1, bf2, g2, b2)
        for m in in_maps:
            m.update(extra)
    res = run_bass_kernel_spmd(_get_nc(lean), in_maps, list(range(8))).results
    return _assemble(res)


# revision 45
# speedup vs baseline: 1.1248x; 1.1248x over previous
"""TRN2 8-core SPMD kernel for nn_DecoderBlock_13443247636967.

Math note (validated to rel err ~1.3e-7 against the fp32 reference):
the reference uses SCALE = head_size**-5 = 2**-30, so every pre-softmax
score satisfies |s| < 4e-8 and the softmax IS the uniform causal
average at fp32 precision.  Attention therefore reduces to a causal
prefix-mean of V, and since prefix-mean commutes with the projections,
AO = prefix_mean(x) @ (Wv_all @ Wo) -- ONE fused [D,D] matrix W_vo
(host-precomputed), applied to causal prefix means of x directly.

Device pipeline per 128-row tile (all GEMMs fp8 DoubleRow, PSUM fp32):
  CxT   = x_tile^T @ (tril*icnt mask)      bf16 PE matmuls (fused
          prefix-mean + transpose in one op)
  AO    = CxT^T @ W_vo_fp8 + rank-2 carry  e5m2 x e4m3 DoubleRow;
          the prefix carry (host colsums @ W_vo) and bo enter as a
          K=2 matmul (icnt/ones rows x cvo/bo rows) into the PSUM
  r1    = AO/SW + x ; LN1 -> N1q (fp8, x16)
  s     = r1*rstd + x  (the -mean*rstd per-row constant provably
          cancels inside LN2, so fp32 N1 is never materialized)
  N1T   = PE transpose of N1q (fp8)
  H^T   = Wf1^T-stationary @ N1T           e4m3 DoubleRow (computing H
          transposed kills the second transpose); relu+quant -> hqT
  z     = hqT^T @ Wf2 / (SH*SW) + s ; LN2 -> out

Sharding: core c = (batch c//2, half c%2) owns 1024 rows, data
parallel; no collectives (carries are host-side prefix colsums).

Emission is software-pipelined (tile j+1 front half before tile j back
half) so the tensor engine never drains and holds its max p-state.
Measured accuracy of this quantization scheme (host sim): 7.6e-3.
"""

import numpy as np
import ml_dtypes

import concourse.bass as bass
import concourse.mybir as mybir
import concourse.tile as tile
from concourse import bacc
from concourse.bass_utils import run_bass_kernel_spmd
from concourse.masks import make_identity

P = 128          # partitions / row-tile height
D = 1024         # model dim
TH = 1024        # sequence rows per core
NT = TH // P     # 8 row tiles
KC = D // P      # 8 contraction chunks
NF = 512         # PSUM half width
NH = D // NF     # 2 column halves
B, T = 4, 2048
EPS = 1e-5
SW = 1024.0      # weight fp8 scale (2**10)
SN = 16.0        # N1 fp8 scale (2**4)
SH = 32.0        # h fp8 scale (2**5)
F32 = mybir.dt.float32
BF16 = mybir.dt.bfloat16
F8E4 = mybir.dt.float8e4
F8E5 = mybir.dt.float8e5
DR = mybir.MatmulPerfMode.DoubleRow
AF = mybir.ActivationFunctionType
OP = mybir.AluOpType


def _build(lean=True):
    nc = bacc.Bacc(
        "TRN2", target_bir_lowering=False, debug=False, num_devices=8
    )
    xf = nc.dram_tensor("xf", [TH, D], F32, kind="ExternalInput").ap()
    xb = nc.dram_tensor("xb", [TH, D], BF16, kind="ExternalInput").ap()
    masks = nc.dram_tensor("masks", [NT, P, P], BF16, kind="ExternalInput").ap()
    cicn = nc.dram_tensor("cicn", [2, NT, P], BF16, kind="ExternalInput").ap()
    crhs = nc.dram_tensor("crhs", [2, NT, D], BF16, kind="ExternalInput").ap()
    Wvo = nc.dram_tensor("Wvo", [P, KC, D], F8E4, kind="ExternalInput").ap()
    Wf1 = nc.dram_tensor("Wf1", [P, KC, D], F8E4, kind="ExternalInput").ap()
    Wf2 = nc.dram_tensor("Wf2", [P, KC, D], F8E4, kind="ExternalInput").ap()
    out = nc.dram_tensor("out", [TH, D], F32, kind="ExternalOutput").ap()
    if not lean:
        vecs = {
            name: nc.dram_tensor(name, [1, D], F32, kind="ExternalInput").ap()
            for name in ["g1", "b1", "bf2", "g2", "b2"]
        }
        bf1T = nc.dram_tensor("bf1T", [P, KC], F32, kind="ExternalInput").ap()

    with tile.TileContext(nc) as tc:
        with tc.tile_pool(name="rows", bufs=1) as rows, \
             tc.tile_pool(name="w", bufs=1) as wpool, \
             tc.tile_pool(name="x", bufs=3) as xpool, \
             tc.tile_pool(name="q", bufs=2) as qpool, \
             tc.tile_pool(name="f", bufs=2) as fpool, \
             tc.tile_pool(name="st", bufs=3) as stat, \
             tc.tile_pool(name="pb", bufs=4, space="PSUM") as pbig, \
             tc.tile_pool(name="pm", bufs=3, space="PSUM") as pmm, \
             tc.tile_pool(name="pt", bufs=1, space="PSUM") as ptp:

            identb = rows.tile([P, P], BF16)
            make_identity(nc, identb)
            eps_t = rows.tile([P, 1], F32)
            nc.vector.memset(eps_t, EPS)
            # constants + weights go on the gpsimd DMA queue so the
            # streaming x tiles (sync queue) are not stuck behind 3MB
            masks_sb = rows.tile([P, NT, P], BF16)
            nc.gpsimd.dma_start(out=masks_sb, in_=masks.rearrange("j p t -> p j t"))
            cicn_sb = rows.tile([2, NT, P], BF16)
            nc.gpsimd.dma_start(out=cicn_sb, in_=cicn)
            crhs_sb = rows.tile([2, NT, D], BF16)
            nc.gpsimd.dma_start(out=crhs_sb, in_=crhs)
            Wvo_sb = wpool.tile([P, KC, D], F8E4, name="Wvo")
            nc.gpsimd.dma_start(out=Wvo_sb, in_=Wvo)
            Wf1_sb = wpool.tile([P, KC, D], F8E4, name="Wf1")
            nc.gpsimd.dma_start(out=Wf1_sb, in_=Wf1)
            Wf2_sb = wpool.tile([P, KC, D], F8E4, name="Wf2")
            nc.gpsimd.dma_start(out=Wf2_sb, in_=Wf2)
            if not lean:
                bc = {
                    name: rows.tile([P, D], F32, name=f"bc_{name}")
                    for name in vecs
                }
                for name in vecs:
                    nc.sync.dma_start(
                        out=bc[name], in_=vecs[name].to_broadcast([P, D])
                    )
                bf1T_sb = rows.tile([P, KC], F32)
                nc.sync.dma_start(out=bf1T_sb, in_=bf1T)

            def stats(src, tag):
                st = stat.tile([P, NH, 6], F32, tag=f"st{tag}")
                for h in range(NH):
                    nc.vector.bn_stats(
                        out=st[:, h, :], in_=src[:, h * NF:(h + 1) * NF]
                    )
                mv = stat.tile([P, 2], F32, tag=f"mv{tag}")
                nc.vector.bn_aggr(out=mv, in_=st)
                rstd = stat.tile([P, 1], F32, tag=f"rs{tag}")
                nc.scalar.activation(
                    out=rstd, in_=mv[:, 1:2], func=AF.Sqrt, bias=eps_t, scale=1.0
                )
                nc.vector.reciprocal(out=rstd, in_=rstd)
                return mv, rstd

            # ---- stage A of tile j: x DMA, CxT, AO, r1, LN1 stats ----
            def stageA(j):
                jsl = slice(j * P, (j + 1) * P)
                xb_t = xpool.tile([P, D], BF16, tag="xb", name="xb")
                nc.sync.dma_start(out=xb_t, in_=xb[jsl, :])
                xf_t = xpool.tile([P, D], F32, tag="xf", name="xf")
                nc.sync.dma_start(out=xf_t, in_=xf[jsl, :])

                # CxT blocks: [d-in-block, t] = prefix-mean^T, two 4-chunk halves
                cxq = qpool.tile([P, KC, P], F8E5, tag="cxq", name="cxq")
                for g in range(2):
                    ps_cx = pbig.tile([P, KC // 2, P], F32, tag="big")
                    for k4 in range(KC // 2):
                        kc = g * 4 + k4
                        nc.tensor.matmul(
                            ps_cx[:, k4, :],
                            lhsT=xb_t[:, kc * P:(kc + 1) * P],
                            rhs=masks_sb[:, j, :],
                            start=True, stop=True,
                        )
                    nc.scalar.activation(
                        out=cxq[:, g * 4:(g + 1) * 4, :], in_=ps_cx,
                        func=AF.Copy, scale=1.0,
                    )

                # AO = rank-2 (carry,bo) + CxT.T @ Wvo  (fp8 DoubleRow)
                r1 = fpool.tile([P, D], F32, tag="r1", name="r1")
                for n in range(NH):
                    nsl = slice(n * NF, (n + 1) * NF)
                    ps = pmm.tile([P, NF], F32, tag="mm")
                    nc.tensor.matmul(
                        ps, lhsT=cicn_sb[:, j, :],
                        rhs=crhs_sb[:, j, nsl],
                        start=True, stop=False,
                    )
                    for k2 in range(KC // 2):
                        nc.tensor.matmul(
                            ps,
                            lhsT=cxq[:, 2 * k2:2 * k2 + 2, :],
                            rhs=Wvo_sb[:, 2 * k2:2 * k2 + 2, nsl],
                            start=False, stop=(k2 == KC // 2 - 1),
                            perf_mode=DR,
                        )
                    # xb carries 16*x, so the tri/rank-2 psum is 16*AO*SW
                    nc.vector.scalar_tensor_tensor(
                        out=r1[:, nsl], in0=ps, scalar=1.0 / (SN * SW),
                        in1=xf_t[:, nsl], op0=OP.mult, op1=OP.add,
                    )

                mv1, rstd1 = stats(r1, "1")
                return r1, xf_t, xb_t, mv1, rstd1

            # ---- stage B of tile j: N1q write + residual s ----
            def stageB(j, actx):
                r1, xf_t, xb_t, mv1, rstd1 = actx
                n1q = qpool.tile([P, D], BF16, tag="n1q", name="n1q")
                s = fpool.tile([P, D], BF16 if lean else F32, tag="s", name="s")
                if lean:
                    rstd16 = stat.tile([P, 1], F32, tag="r16")
                    nc.vector.tensor_scalar_mul(
                        out=rstd16, in0=rstd1, scalar1=SN
                    )
                    mb16 = stat.tile([P, 1], F32, tag="mb16")
                    nc.vector.tensor_scalar(
                        out=mb16, in0=mv1[:, 0:1], scalar1=rstd1,
                        scalar2=-SN, op0=OP.mult, op1=OP.mult,
                    )
                    nc.scalar.activation(
                        out=n1q, in_=r1, func=AF.Identity,
                        bias=mb16, scale=rstd16,
                    )
                    # s = 16*(N1 + x): n1q is 16*N1 (bf16), xb_t is 16*x;
                    # LN2 is invariant to the global *16, and the z STT
                    # scalar carries the matching factor
                    nc.vector.tensor_add(out=s, in0=n1q, in1=xb_t)
                else:
                    mb = stat.tile([P, 1], F32, tag="mb")
                    nc.vector.tensor_scalar(
                        out=mb, in0=mv1[:, 0:1], scalar1=rstd1,
                        scalar2=-1.0, op0=OP.mult, op1=OP.mult,
                    )
                    n1f = fpool.tile([P, D], F32, tag="n1f", name="n1f")
                    nc.scalar.activation(
                        out=n1f, in_=r1, func=AF.Identity, bias=mb, scale=rstd1
                    )
                    nc.vector.tensor_mul(out=n1f, in0=n1f, in1=bc["g1"])
                    nc.vector.tensor_add(out=n1f, in0=n1f, in1=bc["b1"])
                    nc.scalar.activation(
                        out=n1q, in_=n1f, func=AF.Copy, scale=SN
                    )
                    nc.vector.tensor_add(out=s, in0=n1f, in1=xf_t)
                return n1q, s

            # ---- stage C of tile j: N1T, H^T, z, LN2, out ----
            def stageC(j, n1q, s):
                jsl = slice(j * P, (j + 1) * P)
                ps_nt = ptp.tile([P, KC, P], BF16, tag="tp")
                for kc in range(KC):
                    nc.tensor.transpose(
                        ps_nt[:, kc, :], n1q[:, kc * P:(kc + 1) * P], identb
                    )
                n1t = qpool.tile([P, KC, P], F8E4, tag="n1t", name="n1t")
                nc.scalar.activation(
                    out=n1t, in_=ps_nt, func=AF.Copy, scale=1.0
                )

                # H^T[f-in-block, t] per f-block, Wf1 stationary
                hqT = qpool.tile([P, KC, P], F8E4, tag="hqT", name="hqT")
                for g in range(2):
                    ps_ht = pbig.tile([P, KC // 2, P], F32, tag="big")
                    for f4 in range(KC // 2):
                        fb = g * 4 + f4
                        fsl = slice(fb * P, (fb + 1) * P)
                        for k2 in range(KC // 2):
                            nc.tensor.matmul(
                                ps_ht[:, f4, :],
                                lhsT=Wf1_sb[:, 2 * k2:2 * k2 + 2, fsl],
                                rhs=n1t[:, 2 * k2:2 * k2 + 2, :],
                                start=(k2 == 0), stop=(k2 == KC // 2 - 1),
                                perf_mode=DR,
                            )
                    if lean:
                        nc.scalar.activation(
                            out=hqT[:, g * 4:(g + 1) * 4, :], in_=ps_ht,
                            func=AF.Relu, scale=SH / (SN * SW),
                        )
                    else:
                        # SH*relu(H/(SN*SW) + bf1): bias = bf1*SH per f
                        for f4 in range(KC // 2):
                            fb = g * 4 + f4
                            nc.scalar.activation(
                                out=hqT[:, fb, :], in_=ps_ht[:, f4, :],
                                func=AF.Relu, scale=SH / (SN * SW),
                                bias=bf1T_sb[:, fb:fb + 1],
                            )

                zin = fpool.tile([P, D], F32, tag="z", name="z")
                zscal = SN / (SH * SW) if lean else 1.0 / (SH * SW)
                for n in range(NH):
                    nsl = slice(n * NF, (n + 1) * NF)
                    ps = pmm.tile([P, NF], F32, tag="mm")
                    for k2 in range(KC // 2):
                        nc.tensor.matmul(
                            ps,
                            lhsT=hqT[:, 2 * k2:2 * k2 + 2, :],
                            rhs=Wf2_sb[:, 2 * k2:2 * k2 + 2, nsl],
                            start=(k2 == 0), stop=(k2 == KC // 2 - 1),
                            perf_mode=DR,
                        )
                    nc.vector.scalar_tensor_tensor(
                        out=zin[:, nsl], in0=ps, scalar=zscal,
                        in1=s[:, nsl], op0=OP.mult, op1=OP.add,
                    )
                if not lean:
                    nc.vector.tensor_add(out=zin, in0=zin, in1=bc["bf2"])

                mv2, rstd2 = stats(zin, "2")
                mb2 = stat.tile([P, 1], F32, tag="mb2")
                nc.vector.tensor_scalar(
                    out=mb2, in0=mv2[:, 0:1], scalar1=rstd2,
                    scalar2=-1.0, op0=OP.mult, op1=OP.mult,
                )
                o = fpool.tile([P, D], F32, tag="o", name="o")
                nc.scalar.activation(
                    out=o, in_=zin, func=AF.Identity, bias=mb2, scale=rstd2
                )
                if not lean:
                    nc.vector.tensor_mul(out=o, in0=o, in1=bc["g2"])
                    nc.vector.tensor_add(out=o, in0=o, in1=bc["b2"])
                nc.sync.dma_start(out=out[jsl, :], in_=o)

            # software pipeline, emission order A(j+1), C(j), B(j+1):
            # in-order engine queues then never head-of-line block (the
            # ACT queue runs cx(j+1), n1t(j), hqT(j), o(j), n1q(j+1))
            # and the PE always has tile-j+1 matmuls while tile-j LN
            # chains resolve.
            bcur = stageB(0, stageA(0))
            for j in range(NT):
                anext = stageA(j + 1) if j + 1 < NT else None
                stageC(j, *bcur)
                bcur = stageB(j + 1, anext) if j + 1 < NT else None

    nc.compile()
    return nc


_CACHE = {}


def _get_nc(lean=True):
    key = "lean" if lean else "general"
    if key not in _CACHE:
        _CACHE[key] = _build(lean=lean)
    return _CACHE[key]


def _e4(a):
    return np.asarray(a, np.float32).astype(ml_dtypes.float8_e4m3)


def _bf(a):
    return np.asarray(a, np.float32).astype(ml_dtypes.bfloat16)


def _wlayout(w):
    """[D, D] -> [P, KC, D] with element (p, kc, n) = w[kc*P+p, n]."""
    return np.ascontiguousarray(
        np.asarray(w).reshape(KC, P, D).transpose(1, 0, 2)
    )


def _in_maps(x, Wv, Wo, bo, g1, b1, Wf1, bf1, Wf2, bf2, g2, b2):
    x = np.asarray(x, dtype=np.float32)
    Wv_all = np.asarray(Wv, np.float32).transpose(1, 0, 2).reshape(D, D)
    W_vo = Wv_all @ np.asarray(Wo, np.float32)
    base = {
        "Wvo": _wlayout(_e4(W_vo * SW)),
        "Wf1": _wlayout(_e4(np.asarray(Wf1, np.float32) * SW)),
        "Wf2": _wlayout(_e4(np.asarray(Wf2, np.float32) * SW)),
    }
    bo_v = np.asarray(bo, np.float32)

    # per-half masks and carry tables
    # block colsums: cs[b, k] = sum of x[b, :k*P] rows (k = 0..16)
    cs = np.zeros((B, 2 * NT + 1, D), np.float32)
    cs[:, 1:] = np.cumsum(x.reshape(B, 2 * NT, P, D).sum(2), axis=1)

    in_maps = []
    for c in range(8):
        b, half = divmod(c, 2)
        t0b = half * TH
        m = dict(base)
        m["xf"] = np.ascontiguousarray(x[b, t0b:t0b + TH])
        m["xb"] = _bf(m["xf"] * SN)
        tl = np.arange(P, dtype=np.float64)
        masks = np.empty((NT, P, P), np.float32)
        cicn = np.empty((2, NT, P), np.float32)
        crhs = np.empty((2, NT, D), np.float32)
        for j in range(NT):
            t0 = t0b + j * P
            icnt = (1.0 / (t0 + tl + 1.0)).astype(np.float32)
            masks[j] = (tl[:, None] <= tl[None, :]) * icnt[None, :]
            cicn[0, j] = icnt
            cicn[1, j] = 1.0
            carry = cs[b, half * NT + j]
            crhs[0, j] = (carry @ W_vo) * (SN * SW)
            crhs[1, j] = bo_v * (SN * SW)
        m["masks"] = _bf(masks)
        m["cicn"] = _bf(cicn)
        m["crhs"] = _bf(crhs)
        in_maps.append(m)
    return in_maps


def _in_maps_general(g1, b1, bf1, bf2, g2, b2):
    return {
        "g1": np.asarray(g1, np.float32).reshape(1, D),
        "b1": np.asarray(b1, np.float32).reshape(1, D),
        "bf1T": np.ascontiguousarray(
            np.asarray(bf1, np.float32).reshape(KC, P).T * SH
        ),
        "bf2": np.asarray(bf2, np.float32).reshape(1, D),
        "g2": np.asarray(g2, np.float32).reshape(1, D),
        "b2": np.asarray(b2, np.float32).reshape(1, D),
    }


def _assemble(results):
    out = np.empty((B, T, D), np.float32)
    for c in range(8):
        b, half = divmod(c, 2)
        out[b, half * TH:(half + 1) * TH] = results[c]["out"]
    return out


def kernel(x, Wk, Wv, Wo, bo, g1, b1, Wf1, bf1, Wf2, bf2, g2, b2):
    lean = bool(
        not np.any(np.asarray(b1)) and not np.any(np.asarray(bf1))
        and not np.any(np.asarray(bf2)) and not np.any(np.asarray(b2))
        and np.all(np.asarray(g1) == 1.0) and np.all(np.asarray(g2) == 1.0)
    )
    in_maps = _in_maps(x, Wv, Wo, bo, g1, b1, Wf1, bf1, Wf2, bf2, g2, b2)
    if not lean:
        extra = _in_maps_general(g1, b1, bf1, bf2, g2, b2)
        for m in in_maps:
            m.update(extra)
    res = run_bass_kernel_spmd(_get_nc(lean), in_maps, list(range(8))).results
    return _assemble(res)


# revision 52
# speedup vs baseline: 1.2556x; 1.1163x over previous
"""TRN2 8-core SPMD kernel for nn_DecoderBlock_13443247636967.

Math note (validated to rel err ~1.3e-7 against the fp32 reference):
the reference uses SCALE = head_size**-5 = 2**-30, so every pre-softmax
score satisfies |s| < 4e-8 and the softmax IS the uniform causal
average at fp32 precision.  Attention therefore reduces to a causal
prefix-mean of V, and since prefix-mean commutes with the projections,
AO = prefix_mean(x) @ (Wv_all @ Wo) -- ONE fused [D,D] matrix W_vo
(host-precomputed), applied to causal prefix means of x directly.

Device pipeline per 128-row tile (all GEMMs fp8 DoubleRow, PSUM fp32):
  CxT   = x_tile^T @ (tril*icnt mask)      bf16 PE matmuls (fused
          prefix-mean + transpose in one op)
  AO    = CxT^T @ W_vo_fp8 + rank-2 carry  e5m2 x e4m3 DoubleRow;
          the prefix carry (host colsums @ W_vo) and bo enter as a
          K=2 matmul (icnt/ones rows x cvo/bo rows) into the PSUM
  r1    = AO/SW + x ; LN1 -> N1q (fp8, x16)
  s     = r1*rstd + x  (the -mean*rstd per-row constant provably
          cancels inside LN2, so fp32 N1 is never materialized)
  N1T   = PE transpose of N1q (fp8)
  H^T   = Wf1^T-stationary @ N1T           e4m3 DoubleRow (computing H
          transposed kills the second transpose); relu+quant -> hqT
  z     = hqT^T @ Wf2 / (SH*SW) + s ; LN2 -> out

Sharding: core c = (batch c//2, half c%2) owns 1024 rows, data
parallel; no collectives (carries are host-side prefix colsums).

Emission is software-pipelined (tile j+1 front half before tile j back
half) so the tensor engine never drains and holds its max p-state.
Measured accuracy of this quantization scheme (host sim): 7.6e-3.
"""

import numpy as np
import ml_dtypes

import concourse.bass as bass
import concourse.mybir as mybir
import concourse.tile as tile
from concourse import bacc
from concourse.bass_utils import run_bass_kernel_spmd
from concourse.masks import make_identity

P = 128          # partitions / row-tile height
D = 1024         # model dim
TH = 1024        # sequence rows per core
NT = TH // P     # 8 row tiles
KC = D // P      # 8 contraction chunks
NF = 512         # PSUM half width
NH = D // NF     # 2 column halves
B, T = 4, 2048
EPS = 1e-5
SW = 1024.0      # weight fp8 scale (2**10)
SN = 16.0        # N1 fp8 scale (2**4)
SH = 32.0        # h fp8 scale (2**5)
F32 = mybir.dt.float32
BF16 = mybir.dt.bfloat16
F8E4 = mybir.dt.float8e4
F8E5 = mybir.dt.float8e5
DR = mybir.MatmulPerfMode.DoubleRow
AF = mybir.ActivationFunctionType
OP = mybir.AluOpType


def _build(lean=True):
    nc = bacc.Bacc(
        "TRN2", target_bir_lowering=False, debug=False, num_devices=8
    )
    xf = nc.dram_tensor("xf", [TH, D], F32, kind="ExternalInput").ap()
    xb = nc.dram_tensor("xb", [TH, D], BF16, kind="ExternalInput").ap()
    masks = nc.dram_tensor("masks", [NT, P, P], BF16, kind="ExternalInput").ap()
    cicn = nc.dram_tensor("cicn", [2, NT, P], BF16, kind="ExternalInput").ap()
    crhs = nc.dram_tensor("crhs", [2, NT, D], BF16, kind="ExternalInput").ap()
    Wvo = nc.dram_tensor("Wvo", [P, KC, D], F8E4, kind="ExternalInput").ap()
    Wf1 = nc.dram_tensor("Wf1", [P, KC, D], F8E4, kind="ExternalInput").ap()
    Wf2 = nc.dram_tensor("Wf2", [P, KC, D], F8E4, kind="ExternalInput").ap()
    out = nc.dram_tensor("out", [TH, D], F32, kind="ExternalOutput").ap()
    if not lean:
        vecs = {
            name: nc.dram_tensor(name, [1, D], F32, kind="ExternalInput").ap()
            for name in ["g1", "b1", "bf2", "g2", "b2"]
        }
        bf1T = nc.dram_tensor("bf1T", [P, KC], F32, kind="ExternalInput").ap()

    with tile.TileContext(nc) as tc:
        with tc.tile_pool(name="rows", bufs=1) as rows, \
             tc.tile_pool(name="w", bufs=1) as wpool, \
             tc.tile_pool(name="x", bufs=3) as xpool, \
             tc.tile_pool(name="xk", bufs=NT) as xkeep, \
             tc.tile_pool(name="q", bufs=2) as qpool, \
             tc.tile_pool(name="qk", bufs=NT) as qkeep, \
             tc.tile_pool(name="f", bufs=3) as fpool, \
             tc.tile_pool(name="st", bufs=3) as stat, \
             tc.tile_pool(name="pb", bufs=4, space="PSUM") as pbig, \
             tc.tile_pool(name="pm", bufs=3, space="PSUM") as pmm, \
             tc.tile_pool(name="pt", bufs=1, space="PSUM") as ptp:

            identb = rows.tile([P, P], BF16)
            make_identity(nc, identb)
            eps_t = rows.tile([P, 1], F32)
            nc.vector.memset(eps_t, EPS)
            # constants + weights go on the gpsimd DMA queue so the
            # streaming x tiles (sync queue) are not stuck behind 3MB
            masks_sb = rows.tile([P, NT, P], BF16)
            nc.gpsimd.dma_start(out=masks_sb, in_=masks.rearrange("j p t -> p j t"))
            cicn_sb = rows.tile([2, NT, P], BF16)
            nc.gpsimd.dma_start(out=cicn_sb, in_=cicn)
            crhs_sb = rows.tile([2, NT, D], BF16)
            nc.gpsimd.dma_start(out=crhs_sb, in_=crhs)
            Wvo_sb = wpool.tile([P, KC, D], F8E4, name="Wvo")
            nc.gpsimd.dma_start(out=Wvo_sb, in_=Wvo)
            Wf1_sb = wpool.tile([P, KC, D], F8E4, name="Wf1")
            nc.gpsimd.dma_start(out=Wf1_sb, in_=Wf1)
            Wf2_sb = wpool.tile([P, KC, D], F8E4, name="Wf2")
            nc.gpsimd.dma_start(out=Wf2_sb, in_=Wf2)
            if not lean:
                bc = {
                    name: rows.tile([P, D], F32, name=f"bc_{name}")
                    for name in vecs
                }
                for name in vecs:
                    nc.sync.dma_start(
                        out=bc[name], in_=vecs[name].to_broadcast([P, D])
                    )
                bf1T_sb = rows.tile([P, KC], F32)
                nc.sync.dma_start(out=bf1T_sb, in_=bf1T)

            def stats(src, tag):
                st = stat.tile([P, NH, 6], F32, tag=f"st{tag}")
                for h in range(NH):
                    nc.vector.bn_stats(
                        out=st[:, h, :], in_=src[:, h * NF:(h + 1) * NF]
                    )
                mv = stat.tile([P, 2], F32, tag=f"mv{tag}")
                nc.vector.bn_aggr(out=mv, in_=st)
                rstd = stat.tile([P, 1], F32, tag=f"rs{tag}")
                nc.scalar.activation(
                    out=rstd, in_=mv[:, 1:2], func=AF.Sqrt, bias=eps_t, scale=1.0
                )
                nc.vector.reciprocal(out=rstd, in_=rstd)
                return mv, rstd

            # ---- stage A of tile j: x DMA, CxT, AO, r1, LN1 stats ----
            def stageA(j):
                jsl = slice(j * P, (j + 1) * P)
                xb_t = xkeep.tile([P, D], BF16, tag="xb", name="xb")
                nc.sync.dma_start(out=xb_t, in_=xb[jsl, :])
                xf_t = xpool.tile([P, D], F32, tag="xf", name="xf")
                nc.sync.dma_start(out=xf_t, in_=xf[jsl, :])

                # CxT blocks: [d-in-block, t] = prefix-mean^T, two 4-chunk halves
                cxq = qpool.tile([P, KC, P], F8E5, tag="cxq", name="cxq")
                for g in range(2):
                    ps_cx = pbig.tile([P, KC // 2, P], F32, tag="big")
                    for k4 in range(KC // 2):
                        kc = g * 4 + k4
                        nc.tensor.matmul(
                            ps_cx[:, k4, :],
                            lhsT=xb_t[:, kc * P:(kc + 1) * P],
                            rhs=masks_sb[:, j, :],
                            start=True, stop=True,
                        )
                    nc.scalar.activation(
                        out=cxq[:, g * 4:(g + 1) * 4, :], in_=ps_cx,
                        func=AF.Copy, scale=1.0,
                    )

                # AO = rank-2 (carry,bo) + CxT.T @ Wvo  (fp8 DoubleRow)
                r1 = fpool.tile([P, D], F32, tag="r1", name="r1")
                for n in range(NH):
                    nsl = slice(n * NF, (n + 1) * NF)
                    ps = pmm.tile([P, NF], F32, tag="mm")
                    nc.tensor.matmul(
                        ps, lhsT=cicn_sb[:, j, :],
                        rhs=crhs_sb[:, j, nsl],
                        start=True, stop=False,
                    )
                    for k2 in range(KC // 2):
                        nc.tensor.matmul(
                            ps,
                            lhsT=cxq[:, 2 * k2:2 * k2 + 2, :],
                            rhs=Wvo_sb[:, 2 * k2:2 * k2 + 2, nsl],
                            start=False, stop=(k2 == KC // 2 - 1),
                            perf_mode=DR,
                        )
                    # xb carries 16*x, so the tri/rank-2 psum is 16*AO*SW
                    nc.vector.scalar_tensor_tensor(
                        out=r1[:, nsl], in0=ps, scalar=1.0 / (SN * SW),
                        in1=xf_t[:, nsl], op0=OP.mult, op1=OP.add,
                    )

                mv1, rstd1 = stats(r1, "1")
                return r1, xf_t, xb_t, mv1, rstd1

            # ---- stage B of tile j: N1q write + residual s ----
            def stageB(j, actx):
                r1, xf_t, xb_t, mv1, rstd1 = actx
                n1q = qkeep.tile([P, D], BF16, tag="n1q", name="n1q")
                s = None
                if lean:
                    rstd16 = stat.tile([P, 1], F32, tag="r16")
                    nc.vector.tensor_scalar_mul(
                        out=rstd16, in0=rstd1, scalar1=SN
                    )
                    mb16 = stat.tile([P, 1], F32, tag="mb16")
                    nc.vector.tensor_scalar(
                        out=mb16, in0=mv1[:, 0:1], scalar1=rstd1,
                        scalar2=-SN, op0=OP.mult, op1=OP.mult,
                    )
                    nc.scalar.activation(
                        out=n1q, in_=r1, func=AF.Identity,
                        bias=mb16, scale=rstd16,
                    )
                    # lean s = 16*(N1+x) = n1q + xb, computed in stage C
                else:
                    mb = stat.tile([P, 1], F32, tag="mb")
                    nc.vector.tensor_scalar(
                        out=mb, in0=mv1[:, 0:1], scalar1=rstd1,
                        scalar2=-1.0, op0=OP.mult, op1=OP.mult,
                    )
                    n1f = fpool.tile([P, D], F32, tag="n1f", name="n1f")
                    nc.scalar.activation(
                        out=n1f, in_=r1, func=AF.Identity, bias=mb, scale=rstd1
                    )
                    nc.vector.tensor_mul(out=n1f, in0=n1f, in1=bc["g1"])
                    nc.vector.tensor_add(out=n1f, in0=n1f, in1=bc["b1"])
                    nc.scalar.activation(
                        out=n1q, in_=n1f, func=AF.Copy, scale=SN
                    )
                    s = qkeep.tile([P, D], F32, tag="sg", name="sg")
                    nc.vector.tensor_add(out=s, in0=n1f, in1=xf_t)
                return n1q, s, xb_t

            # ---- stage C of tile j: N1T, H^T, z, LN2, out ----
            def stageC(j, n1q, s, xb_t):
                jsl = slice(j * P, (j + 1) * P)
                if lean:
                    # s = 16*(N1 + x): n1q is 16*N1 (bf16), xb_t is 16*x;
                    # LN2 is invariant to the global *16, and the z STT
                    # scalar carries the matching factor
                    s = fpool.tile([P, D], BF16, tag="s", name="s")
                    nc.vector.tensor_add(out=s, in0=n1q, in1=xb_t)
                ps_nt = ptp.tile([P, KC, P], BF16, tag="tp")
                for kc in range(KC):
                    nc.tensor.transpose(
                        ps_nt[:, kc, :], n1q[:, kc * P:(kc + 1) * P], identb
                    )
                n1t = qpool.tile([P, KC, P], F8E4, tag="n1t", name="n1t")
                nc.scalar.activation(
                    out=n1t, in_=ps_nt, func=AF.Copy, scale=1.0
                )

                # H^T[f-in-block, t] per f-block, Wf1 stationary
                hqT = qpool.tile([P, KC, P], F8E4, tag="hqT", name="hqT")
                for g in range(2):
                    ps_ht = pbig.tile([P, KC // 2, P], F32, tag="big")
                    for f4 in range(KC // 2):
                        fb = g * 4 + f4
                        fsl = slice(fb * P, (fb + 1) * P)
                        for k2 in range(KC // 2):
                            nc.tensor.matmul(
                                ps_ht[:, f4, :],
                                lhsT=Wf1_sb[:, 2 * k2:2 * k2 + 2, fsl],
                                rhs=n1t[:, 2 * k2:2 * k2 + 2, :],
                                start=(k2 == 0), stop=(k2 == KC // 2 - 1),
                                perf_mode=DR,
                            )
                    if lean:
                        nc.scalar.activation(
                            out=hqT[:, g * 4:(g + 1) * 4, :], in_=ps_ht,
                            func=AF.Relu, scale=SH / (SN * SW),
                        )
                    else:
                        # SH*relu(H/(SN*SW) + bf1): bias = bf1*SH per f
                        for f4 in range(KC // 2):
                            fb = g * 4 + f4
                            nc.scalar.activation(
                                out=hqT[:, fb, :], in_=ps_ht[:, f4, :],
                                func=AF.Relu, scale=SH / (SN * SW),
                                bias=bf1T_sb[:, fb:fb + 1],
                            )

                zin = fpool.tile([P, D], F32, tag="z", name="z")
                zscal = SN / (SH * SW) if lean else 1.0 / (SH * SW)
                for n in range(NH):
                    nsl = slice(n * NF, (n + 1) * NF)
                    ps = pmm.tile([P, NF], F32, tag="mm")
                    for k2 in range(KC // 2):
                        nc.tensor.matmul(
                            ps,
                            lhsT=hqT[:, 2 * k2:2 * k2 + 2, :],
                            rhs=Wf2_sb[:, 2 * k2:2 * k2 + 2, nsl],
                            start=(k2 == 0), stop=(k2 == KC // 2 - 1),
                            perf_mode=DR,
                        )
                    nc.vector.scalar_tensor_tensor(
                        out=zin[:, nsl], in0=ps, scalar=zscal,
                        in1=s[:, nsl], op0=OP.mult, op1=OP.add,
                    )
                if not lean:
                    nc.vector.tensor_add(out=zin, in0=zin, in1=bc["bf2"])

                mv2, rstd2 = stats(zin, "2")
                mb2 = stat.tile([P, 1], F32, tag="mb2")
                nc.vector.tensor_scalar(
                    out=mb2, in0=mv2[:, 0:1], scalar1=rstd2,
                    scalar2=-1.0, op0=OP.mult, op1=OP.mult,
                )
                o = fpool.tile([P, D], F32, tag="o", name="o")
                nc.scalar.activation(
                    out=o, in_=zin, func=AF.Identity, bias=mb2, scale=rstd2
                )
                if not lean:
                    nc.vector.tensor_mul(out=o, in0=o, in1=bc["g2"])
                    nc.vector.tensor_add(out=o, in0=o, in1=bc["b2"])
                nc.sync.dma_start(out=out[jsl, :], in_=o)

            # phase-major schedule: all attention stages first (tiles
            # independent -> dense overlap), then all FFN stages.  B(j)
            # is emitted after A(j+1) so the in-order ACT queue never
            # blocks a ready cx copy behind a pending LN1 write.
            acts = [stageA(0)]
            bres = []
            for j in range(1, NT):
                acts.append(stageA(j))
                bres.append(stageB(j - 1, acts[j - 1]))
            bres.append(stageB(NT - 1, acts[NT - 1]))
            for j in range(NT):
                stageC(j, *bres[j])

    nc.compile()
    return nc


_CACHE = {}


def _get_nc(lean=True):
    key = "lean" if lean else "general"
    if key not in _CACHE:
        _CACHE[key] = _build(lean=lean)
    return _CACHE[key]


def _e4(a):
    return np.asarray(a, np.float32).astype(ml_dtypes.float8_e4m3)


def _bf(a):
    return np.asarray(a, np.float32).astype(ml_dtypes.bfloat16)


def _wlayout(w):
    """[D, D] -> [P, KC, D] with element (p, kc, n) = w[kc*P+p, n]."""
    return np.ascontiguousarray(
        np.asarray(w).reshape(KC, P, D).transpose(1, 0, 2)
    )


def _in_maps(x, Wv, Wo, bo, g1, b1, Wf1, bf1, Wf2, bf2, g2, b2):
    x = np.asarray(x, dtype=np.float32)
    Wv_all = np.asarray(Wv, np.float32).transpose(1, 0, 2).reshape(D, D)
    W_vo = Wv_all @ np.asarray(Wo, np.float32)
    base = {
        "Wvo": _wlayout(_e4(W_vo * SW)),
        "Wf1": _wlayout(_e4(np.asarray(Wf1, np.float32) * SW)),
        "Wf2": _wlayout(_e4(np.asarray(Wf2, np.float32) * SW)),
    }
    bo_v = np.asarray(bo, np.float32)

    # per-half masks and carry tables
    # block colsums: cs[b, k] = sum of x[b, :k*P] rows (k = 0..16)
    cs = np.zeros((B, 2 * NT + 1, D), np.float32)
    cs[:, 1:] = np.cumsum(x.reshape(B, 2 * NT, P, D).sum(2), axis=1)

    in_maps = []
    for c in range(8):
        b, half = divmod(c, 2)
        t0b = half * TH
        m = dict(base)
        m["xf"] = np.ascontiguousarray(x[b, t0b:t0b + TH])
        m["xb"] = _bf(m["xf"] * SN)
        tl = np.arange(P, dtype=np.float64)
        masks = np.empty((NT, P, P), np.float32)
        cicn = np.empty((2, NT, P), np.float32)
        crhs = np.empty((2, NT, D), np.float32)
        for j in range(NT):
            t0 = t0b + j * P
            icnt = (1.0 / (t0 + tl + 1.0)).astype(np.float32)
            masks[j] = (tl[:, None] <= tl[None, :]) * icnt[None, :]
            cicn[0, j] = icnt
            cicn[1, j] = 1.0
            carry = cs[b, half * NT + j]
            crhs[0, j] = (carry @ W_vo) * (SN * SW)
            crhs[1, j] = bo_v * (SN * SW)
        m["masks"] = _bf(masks)
        m["cicn"] = _bf(cicn)
        m["crhs"] = _bf(crhs)
        in_maps.append(m)
    return in_maps


def _in_maps_general(g1, b1, bf1, bf2, g2, b2):
    return {
        "g1": np.asarray(g1, np.float32).reshape(1, D),
        "b1": np.asarray(b1, np.float32).reshape(1, D),
        "bf1T": np.ascontiguousarray(
            np.asarray(bf1, np.float32).reshape(KC, P).T * SH
        ),
        "bf2": np.asarray(bf2, np.float32).reshape(1, D),
        "g2": np.asarray(g2, np.float32).reshape(1, D),
        "b2": np.asarray(b2, np.float32).reshape(1, D),
    }


def _assemble(results):
    out = np.empty((B, T, D), np.float32)
    for c in range(8):
        b, half = divmod(c, 2)
        out[b, half * TH:(half + 1) * TH] = results[c]["out"]
    return out


def kernel(x, Wk, Wv, Wo, bo, g1, b1, Wf1, bf1, Wf2, bf2, g2, b2):
    lean = bool(
        not np.any(np.asarray(b1)) and not np.any(np.asarray(bf1))
        and not np.any(np.asarray(bf2)) and not np.any(np.asarray(b2))
        and np.all(np.asarray(g1) == 1.0) and np.all(np.asarray(g2) == 1.0)
    )
    in_maps = _in_maps(x, Wv, Wo, bo, g1, b1, Wf1, bf1, Wf2, bf2, g2, b2)
    if not lean:
        extra = _in_maps_general(g1, b1, bf1, bf2, g2, b2)
        for m in in_maps:
            m.update(extra)
    res = run_bass_kernel_spmd(_get_nc(lean), in_maps, list(range(8))).results
    return _assemble(res)


# revision 57
# speedup vs baseline: 1.3723x; 1.0929x over previous
"""TRN2 8-core SPMD kernel for nn_DecoderBlock_13443247636967.

Math note (validated to rel err ~1.3e-7 against the fp32 reference):
the reference uses SCALE = head_size**-5 = 2**-30, so every pre-softmax
score satisfies |s| < 4e-8 and the softmax IS the uniform causal
average at fp32 precision.  Attention therefore reduces to a causal
prefix-mean of V, and since prefix-mean commutes with the projections,
AO = prefix_mean(x) @ (Wv_all @ Wo) -- ONE fused [D,D] matrix W_vo
(host-precomputed), applied to causal prefix means of x directly.

Device pipeline per 128-row tile (all GEMMs fp8 DoubleRow, PSUM fp32):
  CxT   = x_tile^T @ (tril*icnt mask)      bf16 PE matmuls (fused
          prefix-mean + transpose in one op)
  AO    = CxT^T @ W_vo_fp8 + rank-2 carry  e5m2 x e4m3 DoubleRow;
          the prefix carry (host colsums @ W_vo) and bo enter as a
          K=2 matmul (icnt/ones rows x cvo/bo rows) into the PSUM
  r1    = AO/SW + x ; LN1 -> N1q (fp8, x16)
  s     = r1*rstd + x  (the -mean*rstd per-row constant provably
          cancels inside LN2, so fp32 N1 is never materialized)
  N1T   = PE transpose of N1q (fp8)
  H^T   = Wf1^T-stationary @ N1T           e4m3 DoubleRow (computing H
          transposed kills the second transpose); relu+quant -> hqT
  z     = hqT^T @ Wf2 / (SH*SW) + s ; LN2 -> out

Sharding: core c = (batch c//2, half c%2) owns 1024 rows, data
parallel; no collectives (carries are host-side prefix colsums).

Emission is software-pipelined (tile j+1 front half before tile j back
half) so the tensor engine never drains and holds its max p-state.
Measured accuracy of this quantization scheme (host sim): 7.6e-3.
"""

import numpy as np
import ml_dtypes

import concourse.bass as bass
import concourse.mybir as mybir
import concourse.tile as tile
from concourse import bacc
from concourse.bass_utils import run_bass_kernel_spmd
from concourse.masks import make_identity

P = 128          # partitions / row-tile height
D = 1024         # model dim
TH = 1024        # sequence rows per core
NT = TH // P     # 8 row tiles
KC = D // P      # 8 contraction chunks
NF = 512         # PSUM half width
NH = D // NF     # 2 column halves
B, T = 4, 2048
EPS = 1e-5
SW = 1024.0      # weight fp8 scale (2**10)
SN = 16.0        # N1 fp8 scale (2**4)
SH = 32.0        # h fp8 scale (2**5)
F32 = mybir.dt.float32
BF16 = mybir.dt.bfloat16
F8E4 = mybir.dt.float8e4
F8E5 = mybir.dt.float8e5
DR = mybir.MatmulPerfMode.DoubleRow
AF = mybir.ActivationFunctionType
OP = mybir.AluOpType


def _build(lean=True):
    nc = bacc.Bacc(
        "TRN2", target_bir_lowering=False, debug=False, num_devices=8
    )
    xf = nc.dram_tensor("xf", [TH, D], F32, kind="ExternalInput").ap()
    xb = nc.dram_tensor("xb", [TH, D], BF16, kind="ExternalInput").ap()
    masks = nc.dram_tensor("masks", [NT, P, P], BF16, kind="ExternalInput").ap()
    cicn = nc.dram_tensor("cicn", [2, NT, P], BF16, kind="ExternalInput").ap()
    crhs = nc.dram_tensor("crhs", [2, NT, D], BF16, kind="ExternalInput").ap()
    Wvo = nc.dram_tensor("Wvo", [P, KC, D], F8E4, kind="ExternalInput").ap()
    Wf1 = nc.dram_tensor("Wf1", [P, KC, D], F8E4, kind="ExternalInput").ap()
    Wf2 = nc.dram_tensor("Wf2", [P, KC, D], F8E4, kind="ExternalInput").ap()
    out = nc.dram_tensor("out", [TH, D], F32, kind="ExternalOutput").ap()
    if not lean:
        vecs = {
            name: nc.dram_tensor(name, [1, D], F32, kind="ExternalInput").ap()
            for name in ["g1", "b1", "bf2", "g2", "b2"]
        }
        bf1T = nc.dram_tensor("bf1T", [P, KC], F32, kind="ExternalInput").ap()

    with tile.TileContext(nc) as tc:
        with tc.tile_pool(name="rows", bufs=1) as rows, \
             tc.tile_pool(name="w", bufs=1) as wpool, \
             tc.tile_pool(name="x", bufs=3) as xpool, \
             tc.tile_pool(name="xk", bufs=NT) as xkeep, \
             tc.tile_pool(name="q", bufs=2) as qpool, \
             tc.tile_pool(name="qk", bufs=NT) as qkeep, \
             tc.tile_pool(name="f", bufs=3) as fpool, \
             tc.tile_pool(name="st", bufs=3) as stat, \
             tc.tile_pool(name="pb", bufs=4, space="PSUM") as pbig, \
             tc.tile_pool(name="pm", bufs=3, space="PSUM") as pmm, \
             tc.tile_pool(name="pt", bufs=1, space="PSUM") as ptp:

            identb = rows.tile([P, P], BF16)
            make_identity(nc, identb)
            eps_t = rows.tile([P, 1], F32)
            nc.vector.memset(eps_t, EPS)
            # dummy activations: pull the ACT function tables in while
            # the first DMAs stream
            nc.scalar.activation(
                out=eps_t, in_=eps_t, func=AF.Identity, bias=eps_t, scale=0.0
            )
            nc.vector.memset(eps_t, EPS)
            # constants + weights go on the scalar-engine DMA queue so
            # the streaming x tiles (sync queue) are not stuck behind 3MB
            masks_sb = rows.tile([P, NT, P], BF16)
            nc.scalar.dma_start(out=masks_sb, in_=masks.rearrange("j p t -> p j t"))
            cicn_sb = rows.tile([2, NT, P], BF16)
            nc.scalar.dma_start(out=cicn_sb, in_=cicn)
            crhs_sb = rows.tile([2, NT, D], BF16)
            nc.scalar.dma_start(out=crhs_sb, in_=crhs)
            Wvo_sb = wpool.tile([P, KC, D], F8E4, name="Wvo")
            nc.scalar.dma_start(out=Wvo_sb, in_=Wvo)
            Wf1_sb = wpool.tile([P, KC, D], F8E4, name="Wf1")
            nc.scalar.dma_start(out=Wf1_sb, in_=Wf1)
            Wf2_sb = wpool.tile([P, KC, D], F8E4, name="Wf2")
            nc.scalar.dma_start(out=Wf2_sb, in_=Wf2)
            if not lean:
                bc = {
                    name: rows.tile([P, D], F32, name=f"bc_{name}")
                    for name in vecs
                }
                for name in vecs:
                    nc.sync.dma_start(
                        out=bc[name], in_=vecs[name].to_broadcast([P, D])
                    )
                bf1T_sb = rows.tile([P, KC], F32)
                nc.sync.dma_start(out=bf1T_sb, in_=bf1T)

            def stats(src, tag):
                st = stat.tile([P, NH, 6], F32, tag=f"st{tag}")
                for h in range(NH):
                    nc.vector.bn_stats(
                        out=st[:, h, :], in_=src[:, h * NF:(h + 1) * NF]
                    )
                mv = stat.tile([P, 2], F32, tag=f"mv{tag}")
                nc.vector.bn_aggr(out=mv, in_=st)
                rstd = stat.tile([P, 1], F32, tag=f"rs{tag}")
                nc.scalar.activation(
                    out=rstd, in_=mv[:, 1:2], func=AF.Sqrt, bias=eps_t, scale=1.0
                )
                nc.vector.reciprocal(out=rstd, in_=rstd)
                return mv, rstd

            # ---- stage A of tile j: x DMA, CxT, AO, r1, LN1 stats ----
            def stageA(j):
                jsl = slice(j * P, (j + 1) * P)
                xb_t = xkeep.tile([P, D], BF16, tag="xb", name="xb")
                nc.sync.dma_start(out=xb_t, in_=xb[jsl, :])
                xf_t = None
                if not lean:
                    xf_t = xpool.tile([P, D], F32, tag="xf", name="xf")
                    nc.sync.dma_start(out=xf_t, in_=xf[jsl, :])

                # CxT blocks: [d-in-block, t] = prefix-mean^T, two 4-chunk halves
                cxq = qpool.tile([P, KC, P], F8E5, tag="cxq", name="cxq")
                for g in range(2):
                    ps_cx = pbig.tile([P, KC // 2, P], F32, tag="big")
                    for k4 in range(KC // 2):
                        kc = g * 4 + k4
                        nc.tensor.matmul(
                            ps_cx[:, k4, :],
                            lhsT=xb_t[:, kc * P:(kc + 1) * P],
                            rhs=masks_sb[:, j, :],
                            start=True, stop=True,
                        )
                    nc.scalar.activation(
                        out=cxq[:, g * 4:(g + 1) * 4, :], in_=ps_cx,
                        func=AF.Copy, scale=1.0,
                    )

                # AO = rank-2 (carry,bo) + CxT.T @ Wvo  (fp8 DoubleRow)
                r1 = fpool.tile([P, D], F32, tag="r1", name="r1")
                for n in range(NH):
                    nsl = slice(n * NF, (n + 1) * NF)
                    ps = pmm.tile([P, NF], F32, tag="mm")
                    nc.tensor.matmul(
                        ps, lhsT=cicn_sb[:, j, :],
                        rhs=crhs_sb[:, j, nsl],
                        start=True, stop=False,
                    )
                    for k2 in range(KC // 2):
                        nc.tensor.matmul(
                            ps,
                            lhsT=cxq[:, 2 * k2:2 * k2 + 2, :],
                            rhs=Wvo_sb[:, 2 * k2:2 * k2 + 2, nsl],
                            start=False, stop=(k2 == KC // 2 - 1),
                            perf_mode=DR,
                        )
                    # r1 holds 16*(AO + x): xb carries 16*x and the psum
                    # 16*SW*AO; LN1 is scale-invariant so the stats and
                    # the n1q write self-normalize
                    nc.vector.scalar_tensor_tensor(
                        out=r1[:, nsl], in0=ps, scalar=1.0 / SW,
                        in1=xb_t[:, nsl], op0=OP.mult, op1=OP.add,
                    )

                mv1, rstd1 = stats(r1, "1")
                return r1, xf_t, xb_t, mv1, rstd1

            # ---- stage B of tile j: N1q write + residual s ----
            def stageB(j, actx):
                r1, xf_t, xb_t, mv1, rstd1 = actx
                n1q = qkeep.tile([P, D], BF16, tag="n1q", name="n1q")
                s = None
                if lean:
                    rstd16 = stat.tile([P, 1], F32, tag="r16")
                    nc.vector.tensor_scalar_mul(
                        out=rstd16, in0=rstd1, scalar1=SN
                    )
                    mb16 = stat.tile([P, 1], F32, tag="mb16")
                    nc.vector.tensor_scalar(
                        out=mb16, in0=mv1[:, 0:1], scalar1=rstd1,
                        scalar2=-SN, op0=OP.mult, op1=OP.mult,
                    )
                    nc.scalar.activation(
                        out=n1q, in_=r1, func=AF.Identity,
                        bias=mb16, scale=rstd16,
                    )
                    # lean s = 16*(N1+x) = n1q + xb, computed in stage C
                else:
                    mb = stat.tile([P, 1], F32, tag="mb")
                    nc.vector.tensor_scalar(
                        out=mb, in0=mv1[:, 0:1], scalar1=rstd1,
                        scalar2=-1.0, op0=OP.mult, op1=OP.mult,
                    )
                    n1f = fpool.tile([P, D], F32, tag="n1f", name="n1f")
                    nc.scalar.activation(
                        out=n1f, in_=r1, func=AF.Identity, bias=mb, scale=rstd1
                    )
                    nc.vector.tensor_mul(out=n1f, in0=n1f, in1=bc["g1"])
                    nc.vector.tensor_add(out=n1f, in0=n1f, in1=bc["b1"])
                    nc.scalar.activation(
                        out=n1q, in_=n1f, func=AF.Copy, scale=SN
                    )
                    s = qkeep.tile([P, D], F32, tag="sg", name="sg")
                    nc.vector.tensor_add(out=s, in0=n1f, in1=xf_t)
                return n1q, s, xb_t

            # ---- stage C of tile j: N1T, H^T, z, LN2, out ----
            def stageC(j, n1q, s, xb_t):
                jsl = slice(j * P, (j + 1) * P)
                if lean:
                    # s = 16*(N1 + x): n1q is 16*N1 (bf16), xb_t is 16*x;
                    # LN2 is invariant to the global *16, and the z STT
                    # scalar carries the matching factor
                    s = fpool.tile([P, D], BF16, tag="s", name="s")
                    nc.vector.tensor_add(out=s, in0=n1q, in1=xb_t)
                ps_nt = ptp.tile([P, KC, P], BF16, tag="tp")
                for kc in range(KC):
                    nc.tensor.transpose(
                        ps_nt[:, kc, :], n1q[:, kc * P:(kc + 1) * P], identb
                    )
                n1t = qpool.tile([P, KC, P], F8E4, tag="n1t", name="n1t")
                nc.scalar.activation(
                    out=n1t, in_=ps_nt, func=AF.Copy, scale=1.0
                )

                # H^T[f-in-block, t] per f-block, Wf1 stationary
                hqT = qpool.tile([P, KC, P], F8E4, tag="hqT", name="hqT")
                for g in range(2):
                    ps_ht = pbig.tile([P, KC // 2, P], F32, tag="big")
                    for f4 in range(KC // 2):
                        fb = g * 4 + f4
                        fsl = slice(fb * P, (fb + 1) * P)
                        for k2 in range(KC // 2):
                            nc.tensor.matmul(
                                ps_ht[:, f4, :],
                                lhsT=Wf1_sb[:, 2 * k2:2 * k2 + 2, fsl],
                                rhs=n1t[:, 2 * k2:2 * k2 + 2, :],
                                start=(k2 == 0), stop=(k2 == KC // 2 - 1),
                                perf_mode=DR,
                            )
                    if lean:
                        nc.scalar.activation(
                            out=hqT[:, g * 4:(g + 1) * 4, :], in_=ps_ht,
                            func=AF.Relu, scale=SH / (SN * SW),
                        )
                    else:
                        # SH*relu(H/(SN*SW) + bf1): bias = bf1*SH per f
                        for f4 in range(KC // 2):
                            fb = g * 4 + f4
                            nc.scalar.activation(
                                out=hqT[:, fb, :], in_=ps_ht[:, f4, :],
                                func=AF.Relu, scale=SH / (SN * SW),
                                bias=bf1T_sb[:, fb:fb + 1],
                            )

                zin = fpool.tile([P, D], F32, tag="z", name="z")
                zscal = SN / (SH * SW) if lean else 1.0 / (SH * SW)
                for n in range(NH):
                    nsl = slice(n * NF, (n + 1) * NF)
                    ps = pmm.tile([P, NF], F32, tag="mm")
                    for k2 in range(KC // 2):
                        nc.tensor.matmul(
                            ps,
                            lhsT=hqT[:, 2 * k2:2 * k2 + 2, :],
                            rhs=Wf2_sb[:, 2 * k2:2 * k2 + 2, nsl],
                            start=(k2 == 0), stop=(k2 == KC // 2 - 1),
                            perf_mode=DR,
                        )
                    nc.vector.scalar_tensor_tensor(
                        out=zin[:, nsl], in0=ps, scalar=zscal,
                        in1=s[:, nsl], op0=OP.mult, op1=OP.add,
                    )
                if not lean:
                    nc.vector.tensor_add(out=zin, in0=zin, in1=bc["bf2"])

                mv2, rstd2 = stats(zin, "2")
                mb2 = stat.tile([P, 1], F32, tag="mb2")
                nc.vector.tensor_scalar(
                    out=mb2, in0=mv2[:, 0:1], scalar1=rstd2,
                    scalar2=-1.0, op0=OP.mult, op1=OP.mult,
                )
                o = fpool.tile([P, D], F32, tag="o", name="o")
                nc.scalar.activation(
                    out=o, in_=zin, func=AF.Identity, bias=mb2, scale=rstd2
                )
                if not lean:
                    nc.vector.tensor_mul(out=o, in0=o, in1=bc["g2"])
                    nc.vector.tensor_add(out=o, in0=o, in1=bc["b2"])
                nc.sync.dma_start(out=out[jsl, :], in_=o)

            # phase-major schedule: all attention stages first (tiles
            # independent -> dense overlap), then all FFN stages.  B(j)
            # is emitted after A(j+1) so the in-order ACT queue never
            # blocks a ready cx copy behind a pending LN1 write.
            acts = [stageA(0)]
            bres = []
            for j in range(1, NT):
                acts.append(stageA(j))
                bres.append(stageB(j - 1, acts[j - 1]))
            bres.append(stageB(NT - 1, acts[NT - 1]))
            for j in range(NT):
                stageC(j, *bres[j])

    nc.compile()
    return nc


_CACHE = {}


def _get_nc(lean=True):
    key = "lean" if lean else "general"
    if key not in _CACHE:
        _CACHE[key] = _build(lean=lean)
    return _CACHE[key]


def _e4(a):
    return np.asarray(a, np.float32).astype(ml_dtypes.float8_e4m3)


def _bf(a):
    return np.asarray(a, np.float32).astype(ml_dtypes.bfloat16)


def _wlayout(w):
    """[D, D] -> [P, KC, D] with element (p, kc, n) = w[kc*P+p, n]."""
    return np.ascontiguousarray(
        np.asarray(w).reshape(KC, P, D).transpose(1, 0, 2)
    )


def _in_maps(x, Wv, Wo, bo, g1, b1, Wf1, bf1, Wf2, bf2, g2, b2):
    x = np.asarray(x, dtype=np.float32)
    Wv_all = np.asarray(Wv, np.float32).transpose(1, 0, 2).reshape(D, D)
    W_vo = Wv_all @ np.asarray(Wo, np.float32)
    base = {
        "Wvo": _wlayout(_e4(W_vo * SW)),
        "Wf1": _wlayout(_e4(np.asarray(Wf1, np.float32) * SW)),
        "Wf2": _wlayout(_e4(np.asarray(Wf2, np.float32) * SW)),
    }
    bo_v = np.asarray(bo, np.float32)

    # per-half masks and carry tables
    # block colsums: cs[b, k] = sum of x[b, :k*P] rows (k = 0..16)
    cs = np.zeros((B, 2 * NT + 1, D), np.float32)
    cs[:, 1:] = np.cumsum(x.reshape(B, 2 * NT, P, D).sum(2), axis=1)

    in_maps = []
    for c in range(8):
        b, half = divmod(c, 2)
        t0b = half * TH
        m = dict(base)
        m["xf"] = np.ascontiguousarray(x[b, t0b:t0b + TH])
        m["xb"] = _bf(m["xf"] * SN)
        tl = np.arange(P, dtype=np.float64)
        masks = np.empty((NT, P, P), np.float32)
        cicn = np.empty((2, NT, P), np.float32)
        crhs = np.empty((2, NT, D), np.float32)
        for j in range(NT):
            t0 = t0b + j * P
            icnt = (1.0 / (t0 + tl + 1.0)).astype(np.float32)
            masks[j] = (tl[:, None] <= tl[None, :]) * icnt[None, :]
            cicn[0, j] = icnt
            cicn[1, j] = 1.0
            carry = cs[b, half * NT + j]
            crhs[0, j] = (carry @ W_vo) * (SN * SW)
            crhs[1, j] = bo_v * (SN * SW)
        m["masks"] = _bf(masks)
        m["cicn"] = _bf(cicn)
        m["crhs"] = _bf(crhs)
        in_maps.append(m)
    return in_maps


def _in_maps_general(g1, b1, bf1, bf2, g2, b2):
    return {
        "g1": np.asarray(g1, np.float32).reshape(1, D),
        "b1": np.asarray(b1, np.float32).reshape(1, D),
        "bf1T": np.ascontiguousarray(
            np.asarray(bf1, np.float32).reshape(KC, P).T * SH
        ),
        "bf2": np.asarray(bf2, np.float32).reshape(1, D),
        "g2": np.asarray(g2, np.float32).reshape(1, D),
        "b2": np.asarray(b2, np.float32).reshape(1, D),
    }


def _assemble(results):
    out = np.empty((B, T, D), np.float32)
    for c in range(8):
        b, half = divmod(c, 2)
        out[b, half * TH:(half + 1) * TH] = results[c]["out"]
    return out


def kernel(x, Wk, Wv, Wo, bo, g1, b1, Wf1, bf1, Wf2, bf2, g2, b2):
    lean = bool(
        not np.any(np.asarray(b1)) and not np.any(np.asarray(bf1))
        and not np.any(np.asarray(bf2)) and not np.any(np.asarray(b2))
        and np.all(np.asarray(g1) == 1.0) and np.all(np.asarray(g2) == 1.0)
    )
    in_maps = _in_maps(x, Wv, Wo, bo, g1, b1, Wf1, bf1, Wf2, bf2, g2, b2)
    if not lean:
        extra = _in_maps_general(g1, b1, bf1, bf2, g2, b2)
        for m in in_maps:
            m.update(extra)
    res = run_bass_kernel_spmd(_get_nc(lean), in_maps, list(range(8))).results
    return _assemble(res)
